# revision 3
# baseline (speedup 1.0000x reference)
"""BandSplit Trainium2 kernel.

out[b, d, t, k] = sum_{c,w} x[b, c, t, idx[k, w]] * pre_w[k, c*W + w, d] + pre_b[k, d]

Bands are contiguous frequency ranges (mel triangles), so the ragged
gather is a banded sparse matmul. Strategy (8 cores, shard T 8-way):

  * Each core takes t in [core*128, core*128+128) for all 4 batches
    (512 (b,t) rows).
  * x is loaded as (t=128 partitions, 33 chunks x [c*32 + w] cols) --
    the load DMA interleaves the 4 channels at 32-frequency-column
    granularity. PE transpose of each 128x128 block then yields
    xT[chunk] = (4c*32f = 128 partitions, 4b*128t = 512 free): the
    matmul contraction layout.
  * Weights are host-prepacked per (band, 32-aligned f-chunk) incidence
    into 128x128 fp32 tiles (zero rows outside the band) whose row
    order matches xT partitions. Per band: accumulate its incidences
    into one PSUM (d=128, 512) tile, fused bias-add on ScalarE, DMA to
    a device-friendly (K, D, B*128) output. Host reassembles.
"""

import numpy as np

import concourse.bass as bass
import concourse.tile as tile
from concourse import bacc, mybir
from concourse.bass_utils import run_bass_kernel_spmd

F32 = mybir.dt.float32

N_CORES = 8
G = 32  # channel-interleave granularity (f columns per c segment)

_cache = {}


def _band_ranges(indices_pad, mask):
    W = mask.sum(axis=1).astype(np.int64)
    starts = indices_pad[:, 0].astype(np.int64)
    ends = starts + W
    return starts, ends


def _pack_weights(pre_w, starts, ends, Wmax, n_chunks):
    """(NINC, 128, 128) fp32 tiles; rows p = c*G + (f - 32*j) matching xT."""
    K, CW, D = pre_w.shape
    C = CW // Wmax
    pw = pre_w.reshape(K, C, Wmax, D)
    incid = []  # (k, j) pairs in band-major order
    for k in range(K):
        for j in range(int(starts[k]) // G, (int(ends[k]) - 1) // G + 1):
            incid.append((k, j))
    wchunks = np.zeros((len(incid), 128, D), np.float32)
    for i, (k, j) in enumerate(incid):
        f0 = max(int(starts[k]), G * j)
        f1 = min(int(ends[k]), G * (j + 1))
        w0 = f0 - int(starts[k])
        p0 = f0 - G * j
        n = f1 - f0
        for c in range(C):
            wchunks[i, c * G + p0 : c * G + p0 + n, :] = pw[k, c, w0 : w0 + n, :]
    return wchunks, incid


def _build_program(Tc, F, n_chunks, band_chunks):
    """Bass program for one core. Tc = timesteps per core (128)."""
    B, C, K, D = 4, 4, 64, 128
    NF = n_chunks * 128  # xnat2 free size

    nc = bacc.Bacc("TRN2", target_bir_lowering=False, debug=False)
    x_ap = nc.dram_tensor("x", [B, C, Tc, F], F32, kind="ExternalInput").ap()
    w_ap = nc.dram_tensor(
        "w", [sum(len(v) for v in band_chunks), 128, D], F32, kind="ExternalInput"
    ).ap()
    bias_ap = nc.dram_tensor("bias", [D, K], F32, kind="ExternalInput").ap()
    ident_ap = nc.dram_tensor("ident", [128, 128], F32, kind="ExternalInput").ap()
    out_ap = nc.dram_tensor("out", [K, D, B * Tc], F32, kind="ExternalOutput").ap()

    nfull = F // G  # full 32-col f chunks (32)
    with tile.TileContext(nc) as tc:
        from contextlib import ExitStack

        with ExitStack() as ctx:
            const_pool = ctx.enter_context(tc.tile_pool(name="const", bufs=1))
            xnat_pool = ctx.enter_context(tc.tile_pool(name="xnat", bufs=2))
            xt_pool = ctx.enter_context(tc.tile_pool(name="xt", bufs=1))
            w_pool = ctx.enter_context(tc.tile_pool(name="w", bufs=6))
            st_pool = ctx.enter_context(tc.tile_pool(name="st", bufs=3))
            pt_pool = ctx.enter_context(
                tc.tile_pool(name="pt", bufs=3, space="PSUM")
            )
            pm_pool = ctx.enter_context(
                tc.tile_pool(name="pm", bufs=4, space="PSUM")
            )

            ident = const_pool.tile([128, 128], F32, tag="ident")
            nc.sync.dma_start(ident[:], ident_ap[:])
            biasT = const_pool.tile([D, K], F32, tag="bias")
            nc.sync.dma_start(biasT[:], bias_ap[:])

            xT = [
                xt_pool.tile([128, B * Tc], F32, tag=f"xT{j}", name=f"xT{j}")
                for j in range(n_chunks)
            ]

            for b in range(B):
                xn = xnat_pool.tile([Tc, NF], F32, tag="xnat")
                if nfull < n_chunks:
                    # partial last chunk: zero cols, then fill valid ones
                    nc.vector.memset(xn[:, nfull * 128 : n_chunks * 128], 0.0)
                xn3 = xn.rearrange("p (j z) -> p j z", z=128)
                for c in range(C):
                    src = x_ap[b, c][:, : nfull * G].rearrange(
                        "p (j w) -> p j w", w=G
                    )
                    nc.sync.dma_start(
                        xn3[:, :nfull, c * G : (c + 1) * G], src
                    )
                    for fi in range(nfull * G, F):
                        j = fi // G
                        w = fi - G * j
                        nc.sync.dma_start(
                            xn3[:, j, c * G + w : c * G + w + 1],
                            x_ap[b, c][:, fi : fi + 1],
                        )
                for j in range(n_chunks):
                    pt = pt_pool.tile([128, Tc], F32, tag="pt")
                    nc.tensor.transpose(pt[:], xn3[:, j], ident[:])
                    nc.vector.tensor_copy(
                        out=xT[j][:, b * Tc : (b + 1) * Tc], in_=pt[:]
                    )

            inc = 0
            for k in range(64):
                chunks = band_chunks[k]
                pm = pm_pool.tile([D, B * Tc], F32, tag="pm")
                for idx, j in enumerate(chunks):
                    wt = w_pool.tile([128, D], F32, tag="w")
                    nc.sync.dma_start(wt[:], w_ap[inc])
                    nc.tensor.matmul(
                        pm[:],
                        wt[:],
                        xT[j][:],
                        start=(idx == 0),
                        stop=(idx == len(chunks) - 1),
                    )
                    inc += 1
                st = st_pool.tile([D, B * Tc], F32, tag="st")
                nc.scalar.activation(
                    st[:],
                    pm[:],
                    mybir.ActivationFunctionType.Identity,
                    bias=biasT[:, k : k + 1],
                )
                nc.sync.dma_start(out_ap[k], st[:])

    nc.compile()
    return nc


def kernel(x, pre_w, pre_b, indices_pad, mask):
    x = np.asarray(x, np.float32)
    pre_w = np.asarray(pre_w, np.float32)
    pre_b = np.asarray(pre_b, np.float32)
    indices_pad = np.asarray(indices_pad)
    mask_np = np.asarray(mask)

    B, C, T, F = x.shape
    K, CW, D = pre_w.shape
    starts, ends = _band_ranges(indices_pad, mask_np)
    Wmax = CW // C
    n_chunks = (F + G - 1) // G  # 33
    assert T % N_CORES == 0
    Tc = T // N_CORES

    wchunks, incid = _pack_weights(pre_w, starts, ends, Wmax, n_chunks)
    band_chunks = [[] for _ in range(K)]
    for k, j in incid:
        band_chunks[k].append(j)

    key = (B, C, T, F, K, D, tuple(starts.tolist()), tuple(ends.tolist()))
    if key not in _cache:
        _cache[key] = _build_program(Tc, F, n_chunks, band_chunks)
    nc = _cache[key]

    biasT = np.ascontiguousarray(pre_b.T)
    ident = np.eye(128, dtype=np.float32)
    in_maps = []
    for core in range(N_CORES):
        xs = np.ascontiguousarray(x[:, :, core * Tc : (core + 1) * Tc, :])
        in_maps.append({"x": xs, "w": wchunks, "bias": biasT, "ident": ident})

    global _last_in_maps
    _last_in_maps = in_maps
    res = run_bass_kernel_spmd(nc, in_maps, list(range(N_CORES)))

    # per-core out: (K, D, B*Tc) -> full (B, D, T, K)
    arr = np.stack([res.results[i]["out"] for i in range(N_CORES)])
    arr = arr.reshape(N_CORES, K, D, B, Tc)
    out = np.transpose(arr, (3, 2, 0, 4, 1)).reshape(B, D, T, K)
    return np.ascontiguousarray(out)
